# revision 1
# baseline (speedup 1.0000x reference)
"""ChebGraphConv (K=3) on 8 TRN2 NeuronCores.

out = x @ W0 + (Lx) @ W1 + (2L(Lx) - x) @ W2 + bias
    = x @ (W0 - W2) + T1 @ W1 + spmm(U) + bias
where T1 = spmm(x), U = T1 @ (2 W2)   (spmm commutes with right-matmul).

Sharding: destination nodes split 8 ways (N padded 50000 -> 50176 = 8*49*128).
Each core runs two SpMM passes over its ~200k edges as, per 128-dest tile:
  gather source rows (dma_gather, int16 idx -> two half-tables),
  build selector S[e,d] = w_e * (dest_e == d) with one tensor_scalar per
  128-edge chunk, PSUM-accumulate matmuls S.T @ G.
Between passes, U is AllGather'd (two collectives, pipelined behind pass 1).
"""

import numpy as np

import concourse.bass as bass
import concourse.mybir as mybir
import concourse.tile as tile
from concourse import bacc
from concourse.bass_utils import run_bass_kernel_spmd
from concourse.masks import make_identity

N = 50000
NP = 50176          # padded: 8 * 49 * 128
F = 256
P = 128
NCORE = 8
SH = NP // NCORE    # 6272 rows per core
T = SH // P         # 49 dest tiles per core
TA = 24             # tiles whose U rows go to the first AllGather
TB = T - TA         # 25
SHA = TA * P        # 3072
SHB = TB * P        # 3200
XSPLIT = NP // 2    # 25088: phase-1 gather table split (int16 range)

F32 = mybir.dt.float32
BF16 = mybir.dt.bfloat16
I16 = mybir.dt.int16


# ---------------------------------------------------------------- host prep

def _wrap_idx(arr):
    """[n] int16 -> [128, n//16]: 16-partition wrap, replicated for 8 Q7 cores."""
    n = len(arr)
    w16 = arr.reshape(n // 16, 16).T
    return np.tile(w16, (8, 1)).copy()


def _pad_to(arr, n, fill):
    out = np.full(n, fill, arr.dtype)
    out[: len(arr)] = arr
    return out


def prepare(x, edge_row, edge_col, edge_w, weight, bias, mode="full", repeat=1):
    x = np.asarray(x, np.float32)
    edge_row = np.asarray(edge_row, np.int32)
    edge_col = np.asarray(edge_col, np.int32)
    edge_w = np.asarray(edge_w, np.float32)
    weight = np.asarray(weight, np.float32)
    bias = np.asarray(bias, np.float32)

    order = np.argsort(edge_row, kind="stable")
    edge_row = edge_row[order]
    edge_col = edge_col[order]
    edge_w = edge_w[order]

    x_pad = np.zeros((NP, F), np.float32)
    x_pad[:N] = x

    # per-(core,tile) edge ranges over the row-sorted edge list
    bounds = np.searchsorted(edge_row, np.arange(0, NP + 1, P))

    # phase-1 split: col < XSPLIT -> table lo, else hi (idx - XSPLIT)
    # phase-2 split: r = col % SH; r < SHA -> table1 row owner*SHA + r
    #                else table2 row owner*SHB + (r - SHA)
    owner = edge_col // SH
    rloc = edge_col % SH
    m1 = edge_col < XSPLIT
    i1lo = edge_col
    i1hi = edge_col - XSPLIT
    m2 = rloc < SHA
    i2lo = owner * SHA + rloc
    i2hi = owner * SHB + (rloc - SHA)

    dloc = (edge_row % P).astype(np.float32)

    ntile = NCORE * T
    groups = []
    c1l = c1h = c2l = c2h = 1
    for g in range(ntile):
        lo, hi = bounds[g], bounds[g + 1]
        sel = slice(lo, hi)
        msk1 = m1[sel]
        msk2 = m2[sel]
        groups.append((sel, msk1, msk2))
        c1l = max(c1l, -((int(msk1.sum())) // -P))
        c1h = max(c1h, -((int((~msk1).sum())) // -P))
        c2l = max(c2l, -((int(msk2.sum())) // -P))
        c2h = max(c2h, -((int((~msk2).sum())) // -P))

    C1 = c1l + c1h
    C2 = c2l + c2h

    def build_tile(g, phase):
        sel, msk1, msk2 = groups[g]
        msk = msk1 if phase == 1 else msk2
        ilo = (i1lo if phase == 1 else i2lo)[sel]
        ihi = (i1hi if phase == 1 else i2hi)[sel]
        cl, ch = (c1l, c1h) if phase == 1 else (c2l, c2h)
        d_all = dloc[sel]
        w_all = edge_w[sel]
        idx_lo = _pad_to(ilo[msk].astype(np.int16), cl * P, 0)
        idx_hi = _pad_to(ihi[~msk].astype(np.int16), ch * P, 0)
        d_sec = np.concatenate([_pad_to(d_all[msk], cl * P, 0.0),
                                _pad_to(d_all[~msk], ch * P, 0.0)])
        w_sec = np.concatenate([_pad_to(w_all[msk], cl * P, 0.0),
                                _pad_to(w_all[~msk], ch * P, 0.0)])
        ctot = cl + ch
        meta = np.concatenate(
            [d_sec.reshape(ctot, P).T, w_sec.reshape(ctot, P).T], axis=1
        ).astype(np.float32)                       # [128, 2*ctot]
        idx = np.concatenate([_wrap_idx(idx_lo), _wrap_idx(idx_hi)], axis=1)
        return meta, idx

    meta1 = np.empty((NCORE, T, P, 2 * C1), np.float32)
    idx1 = np.empty((NCORE, T, P, C1 * 8), np.int16)
    meta2 = np.empty((NCORE, T, P, 2 * C2), np.float32)
    idx2 = np.empty((NCORE, T, P, C2 * 8), np.int16)
    for c in range(NCORE):
        for t in range(T):
            g = c * T + t
            meta1[c, t], idx1[c, t] = build_tile(g, 1)
            meta2[c, t], idx2[c, t] = build_tile(g, 2)

    # xT tiles: [core, t, 128, 256]; cols k*128+j hold x_pad[tile_row j, feat k*128+p]
    xl = x_pad.reshape(NCORE, T, P, F)
    xtt = np.ascontiguousarray(np.transpose(xl, (0, 1, 3, 2))).reshape(
        NCORE, T, 2, P, P)
    xtt = np.ascontiguousarray(np.transpose(xtt, (0, 1, 3, 2, 4))).reshape(
        NCORE, T, P, F)

    # weights: m0 = W0 - W2, m1 = W1, m2 = 2*W2; wpack[p, (k*3+m)*256 + j]
    wm = np.stack([weight[0] - weight[2], weight[1], 2.0 * weight[2]])  # [3,256,256]
    wpack = np.empty((P, 2 * 3 * F), np.float32)
    for k in range(2):
        for m in range(3):
            wpack[:, (k * 3 + m) * F:(k * 3 + m + 1) * F] = wm[m, k * P:(k + 1) * P, :]

    biasrow = bias.reshape(1, F).astype(np.float32)
    iota = np.broadcast_to(np.arange(P, dtype=np.float32), (P, P)).copy()
    x_lo = np.ascontiguousarray(x_pad[:XSPLIT])
    x_hi = np.ascontiguousarray(x_pad[XSPLIT:])

    nc = build_program(C1, C2, c1l, c1h, c2l, c2h, mode=mode, repeat=repeat)

    in_maps = []
    for c in range(NCORE):
        in_maps.append({
            "x_lo": x_lo, "x_hi": x_hi,
            "meta1": np.ascontiguousarray(meta1[c].reshape(T * P, 2 * C1)),
            "idx1": np.ascontiguousarray(idx1[c].reshape(T * P, C1 * 8)),
            "meta2": np.ascontiguousarray(meta2[c].reshape(T * P, 2 * C2)),
            "idx2": np.ascontiguousarray(idx2[c].reshape(T * P, C2 * 8)),
            "xtt": np.ascontiguousarray(xtt[c].reshape(T * P, F)),
            "wpack": wpack, "biasrow": biasrow, "iota": iota,
        })
    return nc, in_maps


def assemble(results):
    out = np.concatenate([results[c]["out"] for c in range(NCORE)], axis=0)
    return np.ascontiguousarray(out[:N])


# ---------------------------------------------------------------- device

def build_program(C1, C2, c1l, c1h, c2l, c2h, mode="full", repeat=1):
    # mode: "full" | "p1" (phase 1 only, skip collectives+phase 2)
    #       | "p1cc" (phase 1 + collectives, skip phase-2 gathers)
    CMX = max(C1, C2)
    nc = bacc.Bacc("TRN2", target_bir_lowering=False, debug=False,
                   num_devices=NCORE)

    x_lo_d = nc.dram_tensor("x_lo", [XSPLIT, F], F32, kind="ExternalInput")
    x_hi_d = nc.dram_tensor("x_hi", [NP - XSPLIT, F], F32, kind="ExternalInput")
    meta1_d = nc.dram_tensor("meta1", [T * P, 2 * C1], F32, kind="ExternalInput")
    idx1_d = nc.dram_tensor("idx1", [T * P, C1 * 8], I16, kind="ExternalInput")
    meta2_d = nc.dram_tensor("meta2", [T * P, 2 * C2], F32, kind="ExternalInput")
    idx2_d = nc.dram_tensor("idx2", [T * P, C2 * 8], I16, kind="ExternalInput")
    xtt_d = nc.dram_tensor("xtt", [T * P, F], F32, kind="ExternalInput")
    wpack_d = nc.dram_tensor("wpack", [P, 6 * F], F32, kind="ExternalInput")
    bias_d = nc.dram_tensor("biasrow", [1, F], F32, kind="ExternalInput")
    iota_d = nc.dram_tensor("iota", [P, P], F32, kind="ExternalInput")
    out_d = nc.dram_tensor("out", [SH, F], F32, kind="ExternalOutput")

    with tile.TileContext(nc) as tc:
        with tc.tile_pool(name="const", bufs=1) as cp, \
             tc.tile_pool(name="sb", bufs=2) as sb, \
             tc.tile_pool(name="stash", bufs=T) as stash, \
             tc.tile_pool(name="ps", bufs=2, space="PSUM") as ps, \
             tc.tile_pool(name="dram", bufs=1, space="DRAM") as dp:

            iota_t = cp.tile([P, P], F32, tag="iota")
            nc.sync.dma_start(out=iota_t[:], in_=iota_d[:])
            ident = cp.tile([P, P], F32, tag="ident")
            make_identity(nc, ident[:])
            wpk = cp.tile([P, 6 * F], F32, tag="wpk")
            nc.sync.dma_start(out=wpk[:], in_=wpack_d[:])
            bias_t = cp.tile([1, F], F32, tag="bias")
            nc.sync.dma_start(out=bias_t[:], in_=bias_d[:])
            ones_t = cp.tile([1, P], F32, tag="ones")
            nc.vector.memset(ones_t[:], 1.0)


            def w_ap(m, k):
                return wpk[:, (k * 3 + m) * F:(k * 3 + m + 1) * F]

            def spmm_tile(t, cl, ch, meta_d, idx_d, lo_tab, hi_tab, acc_ps):
                """Gather + selector-matmul accumulate into acc_ps [128, F]."""
                C = cl + ch
                meta_t = sb.tile([P, 2 * CMX], F32, tag="meta")
                idx_t = sb.tile([P, CMX * 8], I16, tag="idx")
                nc.sync.dma_start(out=meta_t[:, 0:2 * C],
                                  in_=meta_d[t * P:(t + 1) * P, :])
                nc.sync.dma_start(out=idx_t[:, 0:C * 8],
                                  in_=idx_d[t * P:(t + 1) * P, :])
                g = sb.tile([P, CMX, F], F32, tag="g")
                nc.gpsimd.dma_gather(
                    out_ap=g[:, 0:cl, :], in_ap=lo_tab[:, :],
                    idxs_ap=idx_t[:, 0:cl * 8],
                    num_idxs=cl * P, num_idxs_reg=cl * P, elem_size=F,
                    single_packet=False)
                nc.gpsimd.dma_gather(
                    out_ap=g[:, cl:C, :], in_ap=hi_tab[:, :],
                    idxs_ap=idx_t[:, cl * 8:C * 8],
                    num_idxs=ch * P, num_idxs_reg=ch * P, elem_size=F,
                    single_packet=False)
                for c in range(C):
                    s_t = sb.tile([P, P], F32, tag="s")
                    nc.vector.tensor_scalar(
                        out=s_t[:], in0=iota_t[:],
                        scalar1=meta_t[:, c:c + 1],
                        scalar2=meta_t[:, C + c:C + c + 1],
                        op0=mybir.AluOpType.is_equal, op1=mybir.AluOpType.mult)
                    nc.tensor.matmul(acc_ps[:], lhsT=s_t[:], rhs=g[:, c, :],
                                     start=(c == 0), stop=(c == C - 1))

            for _rep in range(repeat):
                u_a = dp.tile([SHA, F], F32, tag=f"ua{_rep}")
                u_b = dp.tile([SHB, F], F32, tag=f"ub{_rep}")
                u_g1 = dp.tile([NCORE * SHA, F], F32, tag=f"ug1{_rep}",
                               addr_space="Shared")
                u_g2 = dp.tile([NCORE * SHB, F], F32, tag=f"ug2{_rep}",
                               addr_space="Shared")
                # ---------------- phase 1 ----------------
                o1_tiles = []
                for t in range(T):
                    t1_ps = ps.tile([P, F], F32, tag="acc")
                    spmm_tile(t, c1l, c1h, meta1_d, idx1_d, x_lo_d, x_hi_d, t1_ps)

                    t1_sb = sb.tile([P, F], F32, tag="t1sb")
                    nc.scalar.copy(t1_sb[:], t1_ps[:])

                    tp_ps = ps.tile([P, F], F32, tag="tp")
                    nc.tensor.transpose(tp_ps[:, 0:P], t1_sb[:, 0:P], ident[:])
                    nc.tensor.transpose(tp_ps[:, P:F], t1_sb[:, P:F], ident[:])
                    t1T = sb.tile([P, F], F32, tag="t1T")
                    nc.scalar.copy(t1T[:], tp_ps[:])

                    u_ps = ps.tile([P, F], F32, tag="u")
                    nc.tensor.matmul(u_ps[:], lhsT=t1T[:, 0:P], rhs=w_ap(2, 0),
                                     start=True, stop=False)
                    nc.tensor.matmul(u_ps[:], lhsT=t1T[:, P:F], rhs=w_ap(2, 1),
                                     start=False, stop=True)
                    u_sb = sb.tile([P, F], F32, tag="usb")
                    nc.scalar.copy(u_sb[:], u_ps[:])
                    if t < TA:
                        nc.sync.dma_start(out=u_a[t * P:(t + 1) * P, :], in_=u_sb[:])
                    else:
                        nc.sync.dma_start(out=u_b[(t - TA) * P:(t - TA + 1) * P, :],
                                          in_=u_sb[:])

                    xt_t = sb.tile([P, F], F32, tag="xt")
                    nc.sync.dma_start(out=xt_t[:], in_=xtt_d[t * P:(t + 1) * P, :])
                    o_ps = ps.tile([P, F], F32, tag="o")
                    nc.tensor.matmul(o_ps[:], lhsT=xt_t[:, 0:P], rhs=w_ap(0, 0),
                                     start=True, stop=False)
                    nc.tensor.matmul(o_ps[:], lhsT=xt_t[:, P:F], rhs=w_ap(0, 1),
                                     start=False, stop=False)
                    nc.tensor.matmul(o_ps[:], lhsT=t1T[:, 0:P], rhs=w_ap(1, 0),
                                     start=False, stop=False)
                    nc.tensor.matmul(o_ps[:], lhsT=t1T[:, P:F], rhs=w_ap(1, 1),
                                     start=False, stop=False)
                    nc.tensor.matmul(o_ps[:], lhsT=ones_t[:], rhs=bias_t[:],
                                     start=False, stop=True)
                    o1 = stash.tile([P, F], F32, tag="o1")
                    nc.vector.tensor_copy(o1[:], o_ps[:])
                    o1_tiles.append(o1)

                    if t == TA - 1 and mode != "p1":
                        nc.gpsimd.collective_compute(
                            "AllGather", mybir.AluOpType.bypass,
                            replica_groups=[list(range(NCORE))],
                            ins=[u_a[:].opt()], outs=[u_g1[:].opt()])
                if mode != "p1":
                    nc.gpsimd.collective_compute(
                        "AllGather", mybir.AluOpType.bypass,
                        replica_groups=[list(range(NCORE))],
                        ins=[u_b[:].opt()], outs=[u_g2[:].opt()])

                # ---------------- phase 2 ----------------
                for t in range(T):
                    if mode == "full":
                        o2_ps = ps.tile([P, F], F32, tag="acc")
                        spmm_tile(t, c2l, c2h, meta2_d, idx2_d, u_g1, u_g2, o2_ps)
                        fin = sb.tile([P, F], F32, tag="fin")
                        nc.vector.tensor_add(fin[:], o1_tiles[t][:], o2_ps[:])
                    else:
                        fin = sb.tile([P, F], F32, tag="fin")
                        nc.vector.tensor_copy(fin[:], o1_tiles[t][:])
                    nc.sync.dma_start(out=out_d[t * P:(t + 1) * P, :], in_=fin[:])

    nc.compile()
    return nc


# ---------------------------------------------------------------- entry

def kernel(x, edge_row, edge_col, edge_w, weight, bias):
    nc, in_maps = prepare(x, edge_row, edge_col, edge_w, weight, bias)
    res = run_bass_kernel_spmd(nc, in_maps, core_ids=list(range(NCORE)))
    return assemble(res.results)



# revision 12
# speedup vs baseline: 1.8684x; 1.8684x over previous
"""ChebGraphConv (K=3) on 8 TRN2 NeuronCores.

out = x @ W0 + (Lx) @ W1 + (2L(Lx) - x) @ W2 + bias
    = x @ (W0 - W2) + T1 @ W1 + spmm(U) + bias
where T1 = spmm(x), U = T1 @ (2 W2)   (spmm commutes with right-matmul).

Sharding: destination nodes split 8 ways (N padded 50000 -> 50176 = 8*49*128).
Each core runs two SpMM passes over its ~200k edges as, per 128-dest tile:
  gather source rows (dma_gather, int16 idx -> two half-tables, bf16 rows),
  build selector S[e,d] = w_e * (dest_e == d) with one tensor_scalar per
  128-edge chunk (alternating DVE / Pool engines), PSUM-accumulate matmuls.
Phase 1 computes T1 transposed directly (lhsT=G feature halves, rhs=S), so
no PE transposes are needed for the downstream dense matmuls.
Between passes, U (bf16) is AllGather'd (two collectives, first pipelined
behind pass 1).
"""

import numpy as np
import ml_dtypes

import concourse.bass as bass
import concourse.mybir as mybir
import concourse.tile as tile
from concourse import bacc
from concourse.bass_utils import run_bass_kernel_spmd

N = 50000
NP = 50176          # padded: 8 * 49 * 128
F = 256
P = 128
NCORE = 8
SH = NP // NCORE    # 6272 rows per core
T = SH // P         # 49 dest tiles per core
TA = 24             # tiles whose U rows go to the first AllGather
TB = T - TA         # 25
SHA = TA * P        # 3072
SHB = TB * P        # 3200
XSPLIT = NP // 2    # 25088: phase-1 gather table split (int16 range)

F32 = mybir.dt.float32
BF16 = mybir.dt.bfloat16
I16 = mybir.dt.int16
NPBF16 = ml_dtypes.bfloat16


# ---------------------------------------------------------------- host prep

def _wrap_idx(arr):
    """[n] int16 -> [128, n//16]: 16-partition wrap, replicated for 8 Q7 cores."""
    n = len(arr)
    w16 = arr.reshape(n // 16, 16).T
    return np.tile(w16, (8, 1)).copy()


def _pad_to(arr, n, fill):
    out = np.full(n, fill, arr.dtype)
    out[: len(arr)] = arr
    return out


def prepare(x, edge_row, edge_col, edge_w, weight, bias, mode="full", repeat=1,
            pool_every=4, t1_split=False):
    x = np.asarray(x, np.float32)
    edge_row = np.asarray(edge_row, np.int32)
    edge_col = np.asarray(edge_col, np.int32)
    edge_w = np.asarray(edge_w, np.float32)
    weight = np.asarray(weight, np.float32)
    bias = np.asarray(bias, np.float32)

    order = np.argsort(edge_row, kind="stable")
    edge_row = edge_row[order]
    edge_col = edge_col[order]
    edge_w = edge_w[order]

    x_pad = np.zeros((NP, F), np.float32)
    x_pad[:N] = x

    # per-(core,tile) edge ranges over the row-sorted edge list
    bounds = np.searchsorted(edge_row, np.arange(0, NP + 1, P))

    # phase-1 split: col < XSPLIT -> table lo, else hi (idx - XSPLIT)
    # phase-2 split: r = col % SH; r < SHA -> table1 row owner*SHA + r
    #                else table2 row owner*SHB + (r - SHA)
    owner = edge_col // SH
    rloc = edge_col % SH
    m1 = edge_col < XSPLIT
    i1lo = edge_col
    i1hi = edge_col - XSPLIT
    m2 = rloc < SHA
    i2lo = owner * SHA + rloc
    i2hi = owner * SHB + (rloc - SHA)

    dloc = (edge_row % P).astype(np.float32)

    ntile = NCORE * T
    groups = []
    c1l = c1h = c2l = c2h = 1
    for g in range(ntile):
        lo, hi = bounds[g], bounds[g + 1]
        sel = slice(lo, hi)
        msk1 = m1[sel]
        msk2 = m2[sel]
        groups.append((sel, msk1, msk2))
        c1l = max(c1l, -((int(msk1.sum())) // -P))
        c1h = max(c1h, -((int((~msk1).sum())) // -P))
        c2l = max(c2l, -((int(msk2.sum())) // -P))
        c2h = max(c2h, -((int((~msk2).sum())) // -P))

    C1 = c1l + c1h
    C2 = c2l + c2h

    def build_tile(g, phase):
        sel, msk1, msk2 = groups[g]
        msk = msk1 if phase == 1 else msk2
        ilo = (i1lo if phase == 1 else i2lo)[sel]
        ihi = (i1hi if phase == 1 else i2hi)[sel]
        cl, ch = (c1l, c1h) if phase == 1 else (c2l, c2h)
        d_all = dloc[sel]
        w_all = edge_w[sel]
        idx_lo = _pad_to(ilo[msk].astype(np.int16), cl * P, 0)
        idx_hi = _pad_to(ihi[~msk].astype(np.int16), ch * P, 0)
        d_sec = np.concatenate([_pad_to(d_all[msk], cl * P, 0.0),
                                _pad_to(d_all[~msk], ch * P, 0.0)])
        w_sec = np.concatenate([_pad_to(w_all[msk], cl * P, 0.0),
                                _pad_to(w_all[~msk], ch * P, 0.0)])
        ctot = cl + ch
        meta = np.concatenate(
            [d_sec.reshape(ctot, P).T, w_sec.reshape(ctot, P).T], axis=1
        ).astype(np.float32)                       # [128, 2*ctot]
        idx = np.concatenate([_wrap_idx(idx_lo), _wrap_idx(idx_hi)], axis=1)
        return meta, idx

    meta1 = np.empty((NCORE, T, P, 2 * C1), np.float32)
    idx1 = np.empty((NCORE, T, P, C1 * 8), np.int16)
    meta2 = np.empty((NCORE, T, P, 2 * C2), np.float32)
    idx2 = np.empty((NCORE, T, P, C2 * 8), np.int16)
    for c in range(NCORE):
        for t in range(T):
            g = c * T + t
            meta1[c, t], idx1[c, t] = build_tile(g, 1)
            meta2[c, t], idx2[c, t] = build_tile(g, 2)

    # xT tiles: [core, t, 128, 256]; cols k*128+j hold x_pad[tile_row j, feat k*128+p]
    xl = x_pad.reshape(NCORE, T, P, F)
    xtt = np.ascontiguousarray(np.transpose(xl, (0, 1, 3, 2))).reshape(
        NCORE, T, 2, P, P)
    xtt = np.ascontiguousarray(np.transpose(xtt, (0, 1, 3, 2, 4))).reshape(
        NCORE, T, P, F)

    # weights: m0 = W0 - W2, m1 = W1, m2 = 2*W2; wpack[p, (k*3+m)*256 + j]
    wm = np.stack([weight[0] - weight[2], weight[1], 2.0 * weight[2]])  # [3,256,256]
    wpack = np.empty((P, 2 * 3 * F), np.float32)
    for k in range(2):
        for m in range(3):
            wpack[:, (k * 3 + m) * F:(k * 3 + m + 1) * F] = wm[m, k * P:(k + 1) * P, :]

    biasrow = bias.reshape(1, F)
    iota = np.broadcast_to(np.arange(P, dtype=np.float32), (P, P))
    x_lo = x_pad[:XSPLIT]
    x_hi = x_pad[XSPLIT:]

    nc = build_program(C1, C2, c1l, c1h, c2l, c2h, mode=mode, repeat=repeat,
                       pool_every=pool_every, t1_split=t1_split)

    in_maps = []
    for c in range(NCORE):
        in_maps.append({
            "x_lo": x_lo.astype(NPBF16),
            "x_hi": x_hi.astype(NPBF16),
            "meta1": np.ascontiguousarray(meta1[c].reshape(T * P, 2 * C1)),
            "idx1": np.ascontiguousarray(idx1[c].reshape(T * P, C1 * 8)),
            "meta2": np.ascontiguousarray(meta2[c].reshape(T * P, 2 * C2)),
            "idx2": np.ascontiguousarray(idx2[c].reshape(T * P, C2 * 8)),
            "xtt": np.ascontiguousarray(xtt[c].reshape(T * P, F)).astype(NPBF16),
            "wpack": wpack.astype(NPBF16),
            "biasrow": biasrow.astype(NPBF16),
            "iota": iota.astype(NPBF16),
        })
    return nc, in_maps


def assemble(results):
    out = np.concatenate([results[c]["out"] for c in range(NCORE)], axis=0)
    return np.ascontiguousarray(out[:N])


# ---------------------------------------------------------------- device

def build_program(C1, C2, c1l, c1h, c2l, c2h, mode="full", repeat=1,
                  pool_every=4, t1_split=False):
    # mode: "full" | "p1" (phase 1 only, skip collectives+phase 2)
    #       | "p1cc" (phase 1 + collectives, skip phase-2 gathers)
    CMX = max(C1, C2)
    nc = bacc.Bacc("TRN2", target_bir_lowering=False, debug=False,
                   num_devices=NCORE)

    x_lo_d = nc.dram_tensor("x_lo", [XSPLIT, F], BF16, kind="ExternalInput")
    x_hi_d = nc.dram_tensor("x_hi", [NP - XSPLIT, F], BF16, kind="ExternalInput")
    meta1_d = nc.dram_tensor("meta1", [T * P, 2 * C1], F32, kind="ExternalInput")
    idx1_d = nc.dram_tensor("idx1", [T * P, C1 * 8], I16, kind="ExternalInput")
    meta2_d = nc.dram_tensor("meta2", [T * P, 2 * C2], F32, kind="ExternalInput")
    idx2_d = nc.dram_tensor("idx2", [T * P, C2 * 8], I16, kind="ExternalInput")
    xtt_d = nc.dram_tensor("xtt", [T * P, F], BF16, kind="ExternalInput")
    wpack_d = nc.dram_tensor("wpack", [P, 6 * F], BF16, kind="ExternalInput")
    bias_d = nc.dram_tensor("biasrow", [1, F], BF16, kind="ExternalInput")
    iota_d = nc.dram_tensor("iota", [P, P], BF16, kind="ExternalInput")
    out_d = nc.dram_tensor("out", [SH, F], F32, kind="ExternalOutput")

    sel_counter = [0]

    with tile.TileContext(nc) as tc:
        with tc.tile_pool(name="const", bufs=1) as cp, \
             tc.tile_pool(name="sb", bufs=2) as sb, \
             tc.tile_pool(name="sel", bufs=4) as selp, \
             tc.tile_pool(name="stash", bufs=T) as stash, \
             tc.tile_pool(name="ps", bufs=2, space="PSUM") as ps, \
             tc.tile_pool(name="dram", bufs=1, space="DRAM") as dp:

            iota_t = cp.tile([P, P], BF16, tag="iota")
            nc.sync.dma_start(out=iota_t[:], in_=iota_d[:])
            wpk = cp.tile([P, 6 * F], BF16, tag="wpk")
            nc.sync.dma_start(out=wpk[:], in_=wpack_d[:])
            bias_t = cp.tile([1, F], BF16, tag="bias")
            nc.sync.dma_start(out=bias_t[:], in_=bias_d[:])
            ones_t = cp.tile([1, P], BF16, tag="ones")
            nc.vector.memset(ones_t[:], 1.0)

            def w_ap(m, k):
                return wpk[:, (k * 3 + m) * F:(k * 3 + m + 1) * F]

            def build_sel(s_ap, meta_t, c, C):
                """s[e,d] = w_e * (iota[d] == dest_e), alternating engines."""
                k = sel_counter[0]
                sel_counter[0] += 1
                eng = nc.gpsimd if (k % pool_every == pool_every - 1) \
                    else nc.vector
                eng.tensor_scalar(
                    out=s_ap, in0=iota_t[:],
                    scalar1=meta_t[:, c:c + 1],
                    scalar2=meta_t[:, C + c:C + c + 1],
                    op0=mybir.AluOpType.is_equal, op1=mybir.AluOpType.mult)

            def load_tile(t, cl, ch, meta_d, idx_d, lo_tab, hi_tab):
                """DMA meta/idx and gather source rows for one dest tile."""
                C = cl + ch
                meta_t = sb.tile([P, 2 * CMX], F32, tag="meta")
                idx_t = sb.tile([P, CMX * 8], I16, tag="idx")
                nc.sync.dma_start(out=meta_t[:, 0:2 * C],
                                  in_=meta_d[t * P:(t + 1) * P, :])
                nc.sync.dma_start(out=idx_t[:, 0:C * 8],
                                  in_=idx_d[t * P:(t + 1) * P, :])
                g = sb.tile([P, CMX, F], BF16, tag="g")
                nc.gpsimd.dma_gather(
                    out_ap=g[:, 0:cl, :], in_ap=lo_tab[:, :],
                    idxs_ap=idx_t[:, 0:cl * 8],
                    num_idxs=cl * P, num_idxs_reg=cl * P, elem_size=F,
                    single_packet=False)
                nc.gpsimd.dma_gather(
                    out_ap=g[:, cl:C, :], in_ap=hi_tab[:, :],
                    idxs_ap=idx_t[:, cl * 8:C * 8],
                    num_idxs=ch * P, num_idxs_reg=ch * P, elem_size=F,
                    single_packet=False)
                return meta_t, g

            for _rep in range(repeat):
                u_a = dp.tile([SHA, F], BF16, tag=f"ua{_rep}")
                u_b = dp.tile([SHB, F], BF16, tag=f"ub{_rep}")
                u_g1 = dp.tile([NCORE * SHA, F], BF16, tag=f"ug1{_rep}",
                               addr_space="Shared")
                u_g2 = dp.tile([NCORE * SHB, F], BF16, tag=f"ug2{_rep}",
                               addr_space="Shared")
                # ---------------- phase 1 ----------------
                o1_tiles = []
                for t in range(T):
                    C = c1l + c1h
                    meta_t, g = load_tile(t, c1l, c1h, meta1_d, idx1_d,
                                          x_lo_d, x_hi_d)
                    # T1.T accumulated directly: two [128f, 128d] PSUM tiles
                    # (separate banks — interleaved accumulation groups
                    # sharing one bank corrupt results)
                    t1T = sb.tile([P, F], BF16, tag="t1T")
                    t1a_ps = ps.tile([P, P], F32, tag="t1a")
                    t1b_ps = ps.tile([P, P], F32, tag="t1b")
                    halves = [(t1a_ps[:], slice(0, P)),
                              (t1b_ps[:], slice(P, F))]
                    for c in range(C):
                        s_t = selp.tile([P, P], BF16, tag="s")
                        build_sel(s_t[:], meta_t, c, C)
                        for acc_ap, fsl in halves:
                            nc.tensor.matmul(acc_ap, lhsT=g[:, c, fsl],
                                             rhs=s_t[:],
                                             start=(c == 0), stop=(c == C - 1))
                    for acc_ap, fsl in halves:
                        nc.scalar.copy(t1T[:, fsl], acc_ap)

                    u_ps = ps.tile([P, F], F32, tag="uo2")
                    nc.tensor.matmul(u_ps[:], lhsT=t1T[:, 0:P], rhs=w_ap(2, 0),
                                     start=True, stop=False)
                    nc.tensor.matmul(u_ps[:], lhsT=t1T[:, P:F], rhs=w_ap(2, 1),
                                     start=False, stop=True)
                    u_sb = sb.tile([P, F], BF16, tag="usb")
                    nc.scalar.copy(u_sb[:], u_ps[:])
                    if t < TA:
                        nc.sync.dma_start(out=u_a[t * P:(t + 1) * P, :], in_=u_sb[:])
                    else:
                        nc.sync.dma_start(out=u_b[(t - TA) * P:(t - TA + 1) * P, :],
                                          in_=u_sb[:])

                    xt_t = sb.tile([P, F], BF16, tag="xt")
                    nc.sync.dma_start(out=xt_t[:], in_=xtt_d[t * P:(t + 1) * P, :])
                    o_ps = ps.tile([P, F], F32, tag="o")
                    nc.tensor.matmul(o_ps[:], lhsT=xt_t[:, 0:P], rhs=w_ap(0, 0),
                                     start=True, stop=False)
                    nc.tensor.matmul(o_ps[:], lhsT=xt_t[:, P:F], rhs=w_ap(0, 1),
                                     start=False, stop=False)
                    nc.tensor.matmul(o_ps[:], lhsT=t1T[:, 0:P], rhs=w_ap(1, 0),
                                     start=False, stop=False)
                    nc.tensor.matmul(o_ps[:], lhsT=t1T[:, P:F], rhs=w_ap(1, 1),
                                     start=False, stop=False)
                    nc.tensor.matmul(o_ps[:], lhsT=ones_t[:], rhs=bias_t[:],
                                     start=False, stop=True)
                    o1 = stash.tile([P, F], F32, tag="o1")
                    nc.vector.tensor_copy(o1[:], o_ps[:])
                    o1_tiles.append(o1)

                    if t == TA - 1 and mode != "p1":
                        nc.gpsimd.collective_compute(
                            "AllGather", mybir.AluOpType.bypass,
                            replica_groups=[list(range(NCORE))],
                            ins=[u_a[:].opt()], outs=[u_g1[:].opt()])
                if mode != "p1":
                    nc.gpsimd.collective_compute(
                        "AllGather", mybir.AluOpType.bypass,
                        replica_groups=[list(range(NCORE))],
                        ins=[u_b[:].opt()], outs=[u_g2[:].opt()])

                # ---------------- phase 2 ----------------
                for t in range(T):
                    if mode == "full":
                        C = c2l + c2h
                        meta_t, g = load_tile(t, c2l, c2h, meta2_d, idx2_d,
                                              u_g1, u_g2)
                        o2_ps = ps.tile([P, F], F32, tag="uo2")
                        for c in range(C):
                            s_t = selp.tile([P, P], BF16, tag="s")
                            build_sel(s_t[:], meta_t, c, C)
                            nc.tensor.matmul(o2_ps[:], lhsT=s_t[:],
                                             rhs=g[:, c, :],
                                             start=(c == 0), stop=(c == C - 1))
                        fin = sb.tile([P, F], F32, tag="fin")
                        nc.vector.tensor_add(fin[:], o1_tiles[t][:], o2_ps[:])
                    else:
                        fin = sb.tile([P, F], F32, tag="fin")
                        nc.vector.tensor_copy(fin[:], o1_tiles[t][:])
                    nc.sync.dma_start(out=out_d[t * P:(t + 1) * P, :], in_=fin[:])

    nc.compile()
    return nc


# ---------------------------------------------------------------- entry

def kernel(x, edge_row, edge_col, edge_w, weight, bias):
    nc, in_maps = prepare(x, edge_row, edge_col, edge_w, weight, bias)
    res = run_bass_kernel_spmd(nc, in_maps, core_ids=list(range(NCORE)))
    return assemble(res.results)
